# revision 55
# baseline (speedup 1.0000x reference)
"""Fused MLA-with-GQA attention kernel for 8 Trainium2 NeuronCores.

Sharding: 8 cores = 2 (batch) x 4 (kv-head groups). Each core owns one
batch element, 4 query heads and 1 kv head (tensor parallel over heads),
with the kv_lora_rank (512) columns of Wqkv replicated. Each core
computes a partial output  attn_out_g @ Wo[rows_g]  and the host sums
the 4 group partials per batch element.

On-device layout is fully transposed (feature-major) so the whole chain
runs without any transposes:
  C1^T = (X @ W1)^T           lhsT=W1 tile,  rhs=X^T tile
  K^T  = (CKV @ Wk)^T         lhsT=Wk tile,  rhs=CKV^T tile
  V    = CKV @ Wv             lhsT=CKV^T[:, s-sub], rhs=Wv tile
  S^T[k,q] = (Q K^T)^T        lhsT=K^T[:, k-tile], rhs=Q^T
  den[*,q] = sum_k E^T[k,q]   lhsT=ones[128,128],  rhs=E^T  (sum+broadcast)
  O^T[dv,q] = sum_k V E^T     lhsT=V[k-tile],      rhs=E^T
  Y[s,n]  = sum_h O_h^T Wo_h  lhsT=O^T[:, s-sub],  rhs=Wo_h

Precision split (accumulation always fp32 PSUM): the attention-weight
path (X@W1 query columns, K up-projection, Q·K^T scores) runs in fp8
e4m3 with DoubleRow perf mode — the 192-dim nope+rope contraction is
packed as [96, 2, N] so one 0.5 cyc/row matmul computes each score
tile. The value path stays higher precision: X/W1-lora/Wo/V/e in bf16,
V up-projection + O·Wo in bf16/f32r. The softmax denominator is
accumulated on the Vector engine (two alternating SBUF accumulators)
with a single ones-matmul per head for the cross-partition sum.
Causal structure: k-tiles above the diagonal are skipped entirely;
diagonal k-tiles are computed on the column sub-range [p:512] only,
with a triangular mask multiply after exp.
"""

import math
import sys

import numpy as np

for _p in ("/opt/trn_rl_repo", "/root/.axon_site/_ro/trn_rl_repo"):
    if _p not in sys.path:
        try:
            import os

            if os.path.isdir(_p):
                sys.path.insert(0, _p)
        except Exception:
            pass

import concourse.bacc as bacc
import concourse.mybir as mybir
import concourse.tile as tile
from concourse.alu_op_type import AluOpType
from concourse.bass_utils import run_bass_kernel_spmd

# ---- problem constants (hardcoded; kernel.py must be self-contained) ----
HID = 2048
NH = 16
NKV = 4
NG = NH // NKV  # 4 q heads per kv head
LORA = 512
D_ROPE = 64
D_NOPE = 128
D_V = 128
D_QK = D_NOPE + D_ROPE  # 192
B, S = 2, 2048
ROPE_BASE = 10000.0
NCORES = 8

NHC = NG  # heads per core = 4
W1_COLS = NHC * D_QK + LORA  # 4*128 + 128 + 128 + 512 = 1280
SC = 512  # s-chunk width
NCHUNK = S // SC  # 4
KT = 128  # k tile
NKT_TOT = S // KT  # 16
SCALE = 1.0 / math.sqrt(D_QK)

F32 = mybir.dt.float32
F32R = mybir.dt.float32r
BF16 = mybir.dt.bfloat16
FP8 = mybir.dt.float8e4
DR = mybir.MatmulPerfMode.DoubleRow
EXP = mybir.ActivationFunctionType.Exp

_PROGRAM_CACHE = {}


def _build_program(reps: int = 1):
    """reps>1 repeats the whole computation in one NEFF (for timing the
    marginal cost of one repetition, net of dispatch overhead)."""
    nc = bacc.Bacc("TRN2", target_bir_lowering=False, debug=False)

    xt_d = nc.dram_tensor("xt", [HID, S], BF16, kind="ExternalInput").ap()
    xt8_d = nc.dram_tensor("xt8", [HID // 2, 2, S], FP8, kind="ExternalInput").ap()
    w1_d = nc.dram_tensor("w1", [HID, LORA], BF16, kind="ExternalInput").ap()
    w1q8_d = nc.dram_tensor(
        "w1q8", [HID // 2, 2, NHC * D_QK], FP8, kind="ExternalInput"
    ).ap()

    wv_d = nc.dram_tensor("wv", [LORA, D_V], F32, kind="ExternalInput").ap()
    wo_d = nc.dram_tensor("wo", [NHC * D_V, HID], BF16, kind="ExternalInput").ap()
    cos_d = nc.dram_tensor("cosq", [128, S], BF16, kind="ExternalInput").ap()
    sin_d = nc.dram_tensor("sinq", [128, S], BF16, kind="ExternalInput").ap()
    wk8_d = nc.dram_tensor("wk8", [LORA // 2, 2, D_QK], FP8, kind="ExternalInput").ap()
    tri_d = nc.dram_tensor("tri", [128, 128], F32, kind="ExternalInput").ap()
    eye_d = nc.dram_tensor("eye", [128, 128], F32, kind="ExternalInput").ap()
    y_d = nc.dram_tensor("y", [S, HID], BF16, kind="ExternalOutput").ap()

    r = lambda ap: ap.bitcast(F32R)

    from contextlib import ExitStack

    with tile.TileContext(nc) as tc:
        with ExitStack() as ctx:
            constp = ctx.enter_context(tc.tile_pool(name="const", bufs=1))
            wop = ctx.enter_context(tc.tile_pool(name="wo", bufs=1))
            w1p = ctx.enter_context(tc.tile_pool(name="w1s", bufs=1))
            xp = ctx.enter_context(tc.tile_pool(name="x", bufs=1))
            qnp = ctx.enter_context(tc.tile_pool(name="qn", bufs=1))
            ckvp = ctx.enter_context(tc.tile_pool(name="ckv", bufs=1))
            kfp = ctx.enter_context(tc.tile_pool(name="kf", bufs=1))
            vp = ctx.enter_context(tc.tile_pool(name="v", bufs=1))
            ropep = ctx.enter_context(tc.tile_pool(name="rope", bufs=1))
            ep = ctx.enter_context(tc.tile_pool(name="e", bufs=4))
            onp = ctx.enter_context(tc.tile_pool(name="on", bufs=1))
            yp = ctx.enter_context(tc.tile_pool(name="y", bufs=2))
            mmp = ctx.enter_context(tc.tile_pool(name="mm", bufs=5, space="PSUM"))
            denp = ctx.enter_context(tc.tile_pool(name="den", bufs=1, space="PSUM"))
            op_ = ctx.enter_context(tc.tile_pool(name="o", bufs=2, space="PSUM"))
            # ---------------- constants ----------------
            tri_r = constp.tile([128, 128], F32R, tag="tri")
            nc.gpsimd.dma_start(tri_r[:], r(tri_d[:]))
            eye_r = constp.tile([128, 128], F32R, tag="eye")
            nc.gpsimd.dma_start(eye_r[:], r(eye_d[:]))

            ones_f = constp.tile([128, 128], F32, tag="ones_f")
            nc.gpsimd.memset(ones_f[:], 1.0)
            ones_r = constp.tile([128, 128], F32R, tag="ones_r")
            nc.scalar.copy(ones_r[:], ones_f[:])

            # wk: fp8 DoubleRow-packed, 2 tiles of [128, 2, 192] covering
            # lora rows [256t, 256t+256); wv: 4 l-tiles [128, 128] f32r
            wk8_t = []
            wv_t = []
            for t_ in range(2):
                t = constp.tile([128, 2, D_QK], FP8, tag=f"wk8{t_}")
                nc.gpsimd.dma_start(t[:], wk8_d[128 * t_ : 128 * (t_ + 1), 0:2, :])
                wk8_t.append(t)
            for l in range(4):
                t = constp.tile([128, D_V], F32R, tag=f"wv{l}")
                nc.gpsimd.dma_start(t[:], r(wv_d[128 * l : 128 * (l + 1), :]))
                wv_t.append(t)

            # wo resident: per (head, n-block) moving tiles [128, 512], bf16
            wo_t = [[None] * 4 for _ in range(NHC)]
            for h in range(NHC):
                for n in range(4):
                    t = wop.tile([128, 512], BF16, tag=f"wo{h}_{n}")
                    nc.gpsimd.dma_start(
                        t[:], wo_d[128 * h : 128 * (h + 1), 512 * n : 512 * (n + 1)]
                    )
                    wo_t[h][n] = t

            # persistent K state across chunks: fp8 DoubleRow-packed
            # [96, 2, S]; combined dim d = 96*slot + p covers
            # [k_nope(128); k_rope(64)] = 192 rows.
            k8 = kfp.tile([96, 2, S], FP8, tag="k8")
            v_t = [
                vp.tile([128, D_V], BF16, tag=f"v{i}", name=f"v{i}")
                for i in range(NKT_TOT)
            ]

            for rep in range(reps):
              # W1 resident for the whole rep (re-DMA'd once per rep):
              # lora columns in bf16, query columns fp8 DoubleRow-packed
              w1_t = {}
              for si in range(2):
                  for ht in range(16):
                      t = w1p.tile(
                          [128, 256], BF16, tag=f"w1_{ht}_{si}", bufs=1,
                          name=f"w1_{ht}_{si}_{rep}",
                      )
                      eng = nc.scalar if ht < 10 else nc.sync
                      eng.dma_start(
                          t[:],
                          w1_d[128 * ht : 128 * (ht + 1), 256 * si : 256 * (si + 1)],
                      )
                      w1_t[(ht, si)] = t
              w1q8_t = []
              for ht in range(16):
                  t = w1p.tile(
                      [64, 2, NHC * D_QK], FP8, tag=f"w1q8_{ht}", bufs=1,
                      name=f"w1q8_{ht}_{rep}",
                  )
                  eng = nc.scalar if ht < 10 else nc.sync
                  eng.dma_start(t[:], w1q8_d[64 * ht : 64 * (ht + 1), 0:2, :])
                  w1q8_t.append(t)
              # full-width rope tables, once per rep
              cosf = ropep.tile([128, S], BF16, tag="cos", name=f"cos_{rep}")
              nc.gpsimd.dma_start(cosf[:], cos_d[:])
              sinf = ropep.tile([128, S], BF16, tag="sin", name=f"sin_{rep}")
              nc.gpsimd.dma_start(sinf[:], sin_d[:])

              for c in range(NCHUNK):
                s0 = SC * c

                # ---------------- phase A: C1 = X @ W1 (transposed) --------
                x_t = []
                x8_t = []
                for ht in range(16):
                    t = xp.tile([128, SC], BF16, tag=f"x{ht}", bufs=2,
                                name=f"x{ht}_{c}")
                    nc.sync.dma_start(
                        t[:], xt_d[128 * ht : 128 * (ht + 1), s0 : s0 + SC]
                    )
                    x_t.append(t)
                    t8 = xp.tile([64, 2, SC], FP8, tag=f"x8{ht}", bufs=2,
                                  name=f"x8{ht}_{c}")
                    nc.sync.dma_start(
                        t8[:], xt8_d[64 * ht : 64 * (ht + 1), 0:2, s0 : s0 + SC]
                    )
                    x8_t.append(t8)

                q8_t = []
                ckv_t = []
                ckv8_t = [
                    ckvp.tile([128, 2, SC], FP8, tag=f"ckv8_{t_}",
                              name=f"ckv8_{t_}_{c}")
                    for t_ in range(2)
                ]
                qx1_ps = qx2_ps = None
                for j in range(10):
                    ps = mmp.tile([128, SC], F32, tag="mm")
                    if j < 6:
                        # query columns: fp8 DoubleRow at 0.5 cyc/row
                        for ht in range(16):
                            nc.tensor.matmul(
                                ps[:],
                                w1q8_t[ht][0:64, 0:2, 128 * j : 128 * (j + 1)],
                                x8_t[ht][0:64, 0:2, :],
                                start=(ht == 0),
                                stop=(ht == 15),
                                perf_mode=DR,
                            )
                    else:
                        si, off = ((j - 6) // 2, 128 * ((j - 6) % 2))
                        for ht in range(16):
                            nc.tensor.matmul(
                                ps[:],
                                w1_t[(ht, si)][:, off : off + 128],
                                x_t[ht][:],
                                start=(ht == 0),
                                stop=(ht == 15),
                            )
                    if j < 4:
                        # q_nope head j -> fp8 DR tile; slot0 (rows 0:96) is
                        # partition-aligned so DVE writes it directly
                        q8 = qnp.tile([96, 2, SC], FP8, tag=f"q8_{j}",
                                      name=f"q8_{j}_{c}")
                        nc.vector.tensor_copy(q8[0:96, 0, :], ps[0:96, :])
                        qh = qnp.tile([128, SC], FP8, tag=f"qh8_{j}")
                        nc.vector.tensor_copy(qh[96:128, :], ps[96:128, :])
                        nc.gpsimd.dma_start(q8[0:32, 1, :], qh[96:128, :])
                        q8_t.append(q8)
                    elif j == 4:
                        qx1_ps = ps
                    elif j == 5:
                        qx2_ps = ps
                    else:
                        t = ckvp.tile([128, SC], F32R, tag=f"ckv{j - 6}")
                        nc.scalar.copy(t[:], ps[:])
                        jj = j - 6
                        nc.scalar.copy(ckv8_t[jj // 2][0:128, jj % 2, :], ps[:])
                        ckv_t.append(t)

                # rope tables for this chunk
                cos_t = cosf[:, s0 : s0 + SC]
                sin_t = sinf[:, s0 : s0 + SC]

                # ---- Q rope (4 heads batched in 128 partitions) ----
                # All DVE ops full-tile (base partition 0); per-head row
                # extraction done with SB->SB DMAs (free to cross partitions).
                p1 = ropep.tile([128, SC], F32, tag="p1")
                t1 = ropep.tile([128, SC], F32, tag="t1")
                p2 = ropep.tile([128, SC], F32, tag="p2")
                t2 = ropep.tile([128, SC], F32, tag="t2")
                nc.vector.tensor_tensor(p1[:], qx1_ps[:], cos_t, AluOpType.mult)
                nc.vector.tensor_tensor(t1[:], qx2_ps[:], sin_t, AluOpType.mult)
                nc.vector.tensor_tensor(p2[:], qx2_ps[:], cos_t, AluOpType.mult)
                nc.vector.tensor_tensor(t2[:], qx1_ps[:], sin_t, AluOpType.mult)
                o1 = ropep.tile([128, SC], FP8, tag="o1")
                o2 = ropep.tile([128, SC], FP8, tag="o2")
                nc.vector.tensor_tensor(o1[:], p1[:], t1[:], AluOpType.subtract)
                nc.vector.tensor_tensor(o2[:], p2[:], t2[:], AluOpType.add)
                # finish q8 DR packing: rope rows into slot1
                for h in range(NHC):
                    sl = slice(32 * h, 32 * h + 32)
                    nc.gpsimd.dma_start(q8_t[h][32:64, 1, :], o1[sl, :])
                    nc.gpsimd.dma_start(q8_t[h][64:96, 1, :], o2[sl, :])

                # ---------------- K up-projection (fp8 DoubleRow) ---------
                ps_kn = mmp.tile([128, SC], F32, tag="mm")
                for t_ in range(2):
                    nc.tensor.matmul(
                        ps_kn[:],
                        wk8_t[t_][0:128, 0:2, 0:128],
                        ckv8_t[t_][0:128, 0:2, :],
                        start=(t_ == 0), stop=(t_ == 1), perf_mode=DR,
                    )
                # k8 slot0 (rows 0:96) partition-aligned: DVE writes direct
                nc.scalar.copy(k8[0:96, 0, s0 : s0 + SC], ps_kn[0:96, :])
                knh = ropep.tile([128, SC], FP8, tag="knh")
                nc.scalar.copy(knh[96:128, :], ps_kn[96:128, :])

                ps_kr = mmp.tile([64, SC], F32, tag="mm")
                for t_ in range(2):
                    nc.tensor.matmul(
                        ps_kr[:],
                        wk8_t[t_][0:128, 0:2, 128:192],
                        ckv8_t[t_][0:128, 0:2, :],
                        start=(t_ == 0), stop=(t_ == 1), perf_mode=DR,
                    )
                # K rope. kp = [x1*cos; x2*cos], kt = [x1*sin; x2*sin]
                # (cos/sin rows 0:32 == 32:64, so full-tile products work).
                # Swap kt halves via SB->SB DMA, then:
                #   k_rope[0:32]  = kp[0:32]  - kt_swap[0:32]   (= x1 cos - x2 sin)
                #   k_rope[32:64] = kp[32:64] + kt_swap[32:64]  (= x2 cos + x1 sin)
                kp = ropep.tile([64, SC], F32, tag="kp")
                kt_ = ropep.tile([64, SC], F32, tag="kt_")
                kts = ropep.tile([64, SC], F32, tag="kts")
                nc.vector.tensor_tensor(
                    kp[:], ps_kr[:], cosf[0:64, s0 : s0 + SC], AluOpType.mult
                )
                nc.vector.tensor_tensor(
                    kt_[:], ps_kr[:], sinf[0:64, s0 : s0 + SC], AluOpType.mult
                )
                nc.gpsimd.dma_start(kts[0:32, :], kt_[32:64, :])
                nc.gpsimd.dma_start(kts[32:64, :], kt_[0:32, :])
                kr8 = ropep.tile([64, SC], FP8, tag="kr8")
                nc.vector.tensor_tensor(
                    kr8[0:32, :], kp[0:32, :], kts[0:32, :], AluOpType.subtract
                )
                nc.vector.tensor_tensor(
                    kr8[32:64, :], kp[32:64, :], kts[32:64, :], AluOpType.add
                )
                # pack the remaining K slab rows into the fp8 DR layout
                nc.gpsimd.dma_start(k8[0:32, 1, s0 : s0 + SC], knh[96:128, :])
                nc.gpsimd.dma_start(k8[32:96, 1, s0 : s0 + SC], kr8[0:64, :])

                # ---------------- V up-projection ----------------
                # V^T = (CKV @ Wv)^T computed at full rate (free dim 512),
                # then PE-transposed back to V in 128x128 blocks.
                ps_vt = mmp.tile([128, SC], F32, tag="mm", name=f"vt_{c}")
                for l in range(4):
                    nc.tensor.matmul(
                        ps_vt[:], wv_t[l][:], ckv_t[l][:],
                        start=(l == 0), stop=(l == 3),
                    )
                vt_sb = ropep.tile([128, SC], F32R, tag="vt")
                nc.scalar.copy(vt_sb[:], ps_vt[:])
                for ss in range(4):
                    ps_v = mmp.tile([128, 128], F32R, tag="mm", name=f"vtr_{c}_{ss}")
                    nc.tensor.transpose(
                        ps_v[:], vt_sb[:, 128 * ss : 128 * (ss + 1)], eye_r[:]
                    )
                    nc.scalar.copy(v_t[4 * c + ss][:], ps_v[:])

                # ---------------- phase B: attention per head ----------------
                # softmax denominator: e-tiles are accumulated on DVE into
                # one (c==0) or two (c>0, alternating) SBUF accumulators;
                # a single ones-matmul per accumulator at the end of the
                # head does the cross-partition sum + broadcast. This keeps
                # the per-k-tile denominator work off the Tensor engine.
                o_norm = []
                for h in range(NHC):
                    nkt = 4 * c + 4
                    nacc = 1 if c == 0 else 2
                    dacc = [
                        ropep.tile([128, SC], F32R, tag=f"den{a}", bufs=1,
                                   name=f"den{a}_{c}_{h}")
                        for a in range(nacc)
                    ]
                    o_ps = op_.tile([128, SC], F32, tag="o", name=f"o_{c}_{h}")
                    for kt in range(nkt):
                        diag = kt >= 4 * c
                        p = (kt - 4 * c) * 128 if diag else 0
                        s_ps = mmp.tile([128, SC], F32, tag="mm")
                        nc.tensor.matmul(
                            s_ps[:, p:SC],
                            k8[0:96, 0:2, KT * kt : KT * (kt + 1)],
                            q8_t[h][0:96, 0:2, p:SC],
                            start=True,
                            stop=True,
                            perf_mode=DR,
                        )
                        e = ep.tile([128, SC], BF16, tag="e")
                        if diag:
                            tmp = ep.tile([128, 128], F32, tag="ediag", bufs=2,
                                          name=f"ediag_{c}_{h}_{kt}")
                            nc.scalar.activation(
                                tmp[:], s_ps[:, p : p + 128], EXP, scale=SCALE
                            )
                            nc.vector.tensor_tensor(
                                e[:, p : p + 128], tmp[:], tri_r[:], AluOpType.mult
                            )
                            if p + 128 < SC:
                                nc.scalar.activation(
                                    e[:, p + 128 : SC], s_ps[:, p + 128 : SC],
                                    EXP, scale=SCALE,
                                )
                        else:
                            nc.scalar.activation(e[:], s_ps[:], EXP, scale=SCALE)
                        acc = dacc[kt % nacc]
                        if kt < nacc:
                            # first write per accumulator: kt<nacc is always
                            # full-width (c==0,kt=0 has p=0; c>0 kt 0/1 are
                            # off-diagonal)
                            nc.vector.tensor_copy(acc[:], e[:])
                        else:
                            nc.vector.tensor_tensor(
                                acc[:, p:SC], acc[:, p:SC], e[:, p:SC],
                                AluOpType.add,
                            )
                        nc.tensor.matmul(
                            o_ps[:, p:SC],
                            v_t[kt][:],
                            e[:, p:SC],
                            start=(kt == 0),
                            stop=(kt == nkt - 1),
                        )
                    den_ps = denp.tile([128, SC], F32, tag="den")
                    for a in range(nacc):
                        nc.tensor.matmul(
                            den_ps[:], ones_r[:], dacc[a][:],
                            start=(a == 0), stop=(a == nacc - 1),
                        )
                    recip = ropep.tile([128, SC], F32, tag="recip",
                                       name=f"recip_{c}_{h}")
                    nc.vector.reciprocal(recip[:], den_ps[:])
                    on = onp.tile([128, SC], BF16, tag=f"on{h}")
                    nc.vector.tensor_tensor(on[:], o_ps[:], recip[:], AluOpType.mult)
                    o_norm.append(on)

                # ---------------- phase C: Y = O @ Wo (partial) -------------
                # y staged [128, 1024] (2 n-blocks) -> bigger output DMAs,
                # issued on the SWDGE ring to keep the HWDGE rings free
                for ss in range(4):
                    for np_ in range(2):
                        y_sb = yp.tile([128, 1024], BF16, tag="y", name=f"y_{c}_{ss}_{np_}")
                        for nn in range(2):
                            n = 2 * np_ + nn
                            y_ps = mmp.tile([128, 512], F32, tag="mm", name=f"yps_{c}_{ss}_{n}")
                            for h in range(NHC):
                                nc.tensor.matmul(
                                    y_ps[:],
                                    o_norm[h][:, 128 * ss : 128 * (ss + 1)],
                                    wo_t[h][n][:],
                                    start=(h == 0),
                                    stop=(h == NHC - 1),
                                )
                            nc.scalar.copy(y_sb[:, 512 * nn : 512 * (nn + 1)], y_ps[:])
                        nc.gpsimd.dma_start(
                            y_d[s0 + 128 * ss : s0 + 128 * (ss + 1),
                                1024 * np_ : 1024 * (np_ + 1)],
                            y_sb[:],
                        )

    nc.compile()
    return nc


def _pack_dr_rows(a, half=64):
    """[rows, cols] -> [rows//2, 2, cols]: within each 2*half-row block,
    row d = half*slot + p lands at (block*half + p, slot)."""
    rows, cols = a.shape
    t = a.reshape(rows // (2 * half), 2, half, cols)
    t = np.swapaxes(t, 1, 2)
    return np.ascontiguousarray(t.reshape(rows // 2, 2, cols))


def _host_inputs(hidden_states, Wqkv, Wk_up, Wv_up, Wo):
    """Build the 8 per-core input maps."""
    bf16 = mybir.dt.np(mybir.dt.bfloat16)
    f8 = mybir.dt.np(mybir.dt.float8e4)
    inv_freq = 1.0 / (ROPE_BASE ** (np.arange(0, D_ROPE, 2, dtype=np.float32) / D_ROPE))
    t = np.arange(S, dtype=np.float32)
    freqs = np.outer(t, inv_freq)  # [S, 32]
    cosq = np.ascontiguousarray(np.tile(np.cos(freqs).T, (4, 1))).astype(bf16)
    sinq = np.ascontiguousarray(np.tile(np.sin(freqs).T, (4, 1))).astype(bf16)
    tri = np.triu(np.ones((128, 128), dtype=np.float32))
    eye = np.eye(128, dtype=np.float32)

    lora_cols = Wqkv[:, NH * D_QK :]  # [HID, LORA]
    in_maps = []
    per_g = {}
    for g in range(NKV):
        nopes, x1s, x2s = [], [], []
        for h in range(NHC):
            H = NHC * g + h
            base = H * D_QK
            nopes.append(Wqkv[:, base : base + D_NOPE])
            x1s.append(Wqkv[:, base + D_NOPE : base + D_NOPE + 32])
            x2s.append(Wqkv[:, base + D_NOPE + 32 : base + D_QK])
        w1 = np.ascontiguousarray(lora_cols).astype(bf16)
        w1q8 = _pack_dr_rows(
            np.concatenate(nopes + x1s + x2s, axis=1).astype(np.float32)
        ).astype(f8)
        wk8 = _pack_dr_rows(
            np.concatenate(
                [
                    Wk_up[:, g * D_QK : g * D_QK + D_NOPE],
                    Wk_up[:, g * D_QK + D_NOPE : g * D_QK + D_NOPE + 32],
                    Wk_up[:, g * D_QK + D_NOPE + 32 : (g + 1) * D_QK],
                ],
                axis=1,
            ).astype(np.float32),
            half=128,
        ).astype(f8)
        wv = np.ascontiguousarray(Wv_up[:, g * D_V : (g + 1) * D_V]).astype(np.float32)
        wo = np.ascontiguousarray(Wo[g * NHC * D_V : (g + 1) * NHC * D_V, :]).astype(
            bf16
        )
        per_g[g] = (w1, w1q8, wk8, wv, wo)

    xt_b = {}
    xt8_b = {}
    for b in range(B):
        xt_f32 = np.ascontiguousarray(hidden_states[b].T).astype(np.float32)
        xt_b[b] = xt_f32.astype(bf16)
        xt8_b[b] = _pack_dr_rows(xt_f32).astype(f8)

    for core in range(NCORES):
        b, g = core // NKV, core % NKV
        w1, w1q8, wk8, wv, wo = per_g[g]
        in_maps.append(
            {
                "xt": xt_b[b],
                "xt8": xt8_b[b],
                "w1": w1,
                "w1q8": w1q8,
                "wk8": wk8,
                "wv": wv,
                "wo": wo,
                "cosq": cosq,
                "sinq": sinq,
                "tri": tri,
                "eye": eye,
            }
        )
    return in_maps


def kernel(hidden_states, Wqkv, Wk_up, Wv_up, Wo):
    hidden_states = np.asarray(hidden_states, dtype=np.float32)
    Wqkv = np.asarray(Wqkv, dtype=np.float32)
    Wk_up = np.asarray(Wk_up, dtype=np.float32)
    Wv_up = np.asarray(Wv_up, dtype=np.float32)
    Wo = np.asarray(Wo, dtype=np.float32)

    if "nc" not in _PROGRAM_CACHE:
        _PROGRAM_CACHE["nc"] = _build_program()
    nc = _PROGRAM_CACHE["nc"]

    in_maps = _host_inputs(hidden_states, Wqkv, Wk_up, Wv_up, Wo)
    res = run_bass_kernel_spmd(nc, in_maps, list(range(NCORES)))

    out = np.zeros((B, S, HID), dtype=np.float32)
    for core in range(NCORES):
        b = core // NKV
        out[b] += res.results[core]["y"].astype(np.float32)
    return out


if __name__ == "__main__":
    rng = np.random.default_rng(0)
    hs = rng.standard_normal((B, S, HID)).astype(np.float32)
    wqkv = rng.standard_normal((HID, NH * D_QK + LORA)).astype(np.float32) * 0.02
    wk = rng.standard_normal((LORA, NKV * D_QK)).astype(np.float32) * 0.04
    wv = rng.standard_normal((LORA, NKV * D_V)).astype(np.float32) * 0.04
    wo = rng.standard_normal((NH * D_V, HID)).astype(np.float32) * 0.02
    y = kernel(hs, wqkv, wk, wv, wo)
    print("kernel output", y.shape, y.dtype, float(np.abs(y).max()))



# revision 67
# speedup vs baseline: 5.3941x; 5.3941x over previous
"""Fused MLA-with-GQA attention kernel for 8 Trainium2 NeuronCores.

Sharding: 8 cores = 2 (batch) x 4 (kv-head groups). Each core owns one
batch element, 4 query heads and 1 kv head (tensor parallel over heads),
with the kv_lora_rank (512) columns of Wqkv replicated. Each core
computes a partial output  attn_out_g @ Wo[rows_g]  and the host sums
the 4 group partials per batch element.

On-device layout is fully transposed (feature-major) so the whole chain
runs without any transposes:
  C1^T = (X @ W1)^T           lhsT=W1 tile,  rhs=X^T tile
  K^T  = (CKV @ Wk)^T         lhsT=Wk tile,  rhs=CKV^T tile
  V    = CKV @ Wv             lhsT=CKV^T[:, s-sub], rhs=Wv tile
  S^T[k,q] = (Q K^T)^T        lhsT=K^T[:, k-tile], rhs=Q^T
  den[*,q] = sum_k E^T[k,q]   lhsT=ones[128,128],  rhs=E^T  (sum+broadcast)
  O^T[dv,q] = sum_k V E^T     lhsT=V[k-tile],      rhs=E^T
  Y[s,n]  = sum_h O_h^T Wo_h  lhsT=O^T[:, s-sub],  rhs=Wo_h

Precision split (accumulation always fp32 PSUM): the attention-weight
path (X@W1 query columns, K up-projection, Q·K^T scores) runs in fp8
e4m3 with DoubleRow perf mode — the 192-dim nope+rope contraction is
packed as [96, 2, N] so one 0.5 cyc/row matmul computes each score
tile. The value path stays higher precision: X/W1-lora/Wo/V/e in bf16,
V up-projection + O·Wo in bf16/f32r. The softmax denominator is
accumulated on the Vector engine (two alternating SBUF accumulators)
with a single ones-matmul per head for the cross-partition sum.
Causal structure: k-tiles above the diagonal are skipped entirely;
diagonal k-tiles are computed on the column sub-range [p:512] only,
with a triangular mask multiply after exp.
"""

import math
import sys

import numpy as np

for _p in ("/opt/trn_rl_repo", "/root/.axon_site/_ro/trn_rl_repo"):
    if _p not in sys.path:
        try:
            import os

            if os.path.isdir(_p):
                sys.path.insert(0, _p)
        except Exception:
            pass

import concourse.bacc as bacc
import concourse.mybir as mybir
import concourse.tile as tile
from concourse.alu_op_type import AluOpType
from concourse.bass_utils import run_bass_kernel_spmd

# ---- problem constants (hardcoded; kernel.py must be self-contained) ----
HID = 2048
NH = 16
NKV = 4
NG = NH // NKV  # 4 q heads per kv head
LORA = 512
D_ROPE = 64
D_NOPE = 128
D_V = 128
D_QK = D_NOPE + D_ROPE  # 192
B, S = 2, 2048
ROPE_BASE = 10000.0
NCORES = 8

NHC = NG  # heads per core = 4
W1_COLS = NHC * D_QK + LORA  # 4*128 + 128 + 128 + 512 = 1280
SC = 512  # s-chunk width
NCHUNK = S // SC  # 4
KT = 128  # k tile
NKT_TOT = S // KT  # 16
SCALE = 1.0 / math.sqrt(D_QK)

F32 = mybir.dt.float32
F32R = mybir.dt.float32r
BF16 = mybir.dt.bfloat16
FP8 = mybir.dt.float8e4
DR = mybir.MatmulPerfMode.DoubleRow
EXP = mybir.ActivationFunctionType.Exp

_PROGRAM_CACHE = {}


def _build_program(reps: int = 1):
    """reps>1 repeats the whole computation in one NEFF (for timing the
    marginal cost of one repetition, net of dispatch overhead)."""
    nc = bacc.Bacc("TRN2", target_bir_lowering=False, debug=False)

    xt_d = nc.dram_tensor("xt", [HID, S], BF16, kind="ExternalInput").ap()
    xt8_d = nc.dram_tensor("xt8", [HID // 2, 2, S], FP8, kind="ExternalInput").ap()
    w1_d = nc.dram_tensor("w1", [HID, LORA], BF16, kind="ExternalInput").ap()
    w1q8_d = nc.dram_tensor(
        "w1q8", [HID // 2, 2, NHC * D_QK], FP8, kind="ExternalInput"
    ).ap()

    wv_d = nc.dram_tensor("wv", [LORA, D_V], F32, kind="ExternalInput").ap()
    wo_d = nc.dram_tensor("wo", [NHC * D_V, HID], BF16, kind="ExternalInput").ap()
    cos_d = nc.dram_tensor("cosq", [128, S], BF16, kind="ExternalInput").ap()
    sin_d = nc.dram_tensor("sinq", [128, S], BF16, kind="ExternalInput").ap()
    wk8_d = nc.dram_tensor("wk8", [LORA // 2, 2, D_QK], FP8, kind="ExternalInput").ap()
    tri_d = nc.dram_tensor("tri", [128, 128], F32, kind="ExternalInput").ap()
    eye_d = nc.dram_tensor("eye", [128, 128], F32, kind="ExternalInput").ap()
    y_d = nc.dram_tensor("y", [S, HID], BF16, kind="ExternalOutput").ap()

    r = lambda ap: ap.bitcast(F32R)

    from contextlib import ExitStack

    with tile.TileContext(nc) as tc:
        with ExitStack() as ctx:
            constp = ctx.enter_context(tc.tile_pool(name="const", bufs=1))
            wop = ctx.enter_context(tc.tile_pool(name="wo", bufs=1))
            w1p = ctx.enter_context(tc.tile_pool(name="w1s", bufs=1))
            xp = ctx.enter_context(tc.tile_pool(name="x", bufs=1))
            qnp = ctx.enter_context(tc.tile_pool(name="qn", bufs=1))
            ckvp = ctx.enter_context(tc.tile_pool(name="ckv", bufs=1))
            kfp = ctx.enter_context(tc.tile_pool(name="kf", bufs=1))
            vp = ctx.enter_context(tc.tile_pool(name="v", bufs=1))
            ropep = ctx.enter_context(tc.tile_pool(name="rope", bufs=1))
            ep = ctx.enter_context(tc.tile_pool(name="e", bufs=4))
            onp = ctx.enter_context(tc.tile_pool(name="on", bufs=1))
            yp = ctx.enter_context(tc.tile_pool(name="y", bufs=2))
            mmp = ctx.enter_context(tc.tile_pool(name="mm", bufs=5, space="PSUM"))
            denp = ctx.enter_context(tc.tile_pool(name="den", bufs=1, space="PSUM"))
            op_ = ctx.enter_context(tc.tile_pool(name="o", bufs=2, space="PSUM"))
            # ---------------- constants ----------------
            tri_r = constp.tile([128, 128], F32R, tag="tri")
            nc.gpsimd.dma_start(tri_r[:], r(tri_d[:]))
            eye_r = constp.tile([128, 128], F32R, tag="eye")
            nc.gpsimd.dma_start(eye_r[:], r(eye_d[:]))

            ones_f = constp.tile([128, 128], F32, tag="ones_f")
            nc.gpsimd.memset(ones_f[:], 1.0)
            ones_r = constp.tile([128, 128], F32R, tag="ones_r")
            nc.scalar.copy(ones_r[:], ones_f[:])

            # wk: fp8 DoubleRow-packed, 2 tiles of [128, 2, 192] covering
            # lora rows [256t, 256t+256); wv: 4 l-tiles [128, 128] f32r
            wk8_t = []
            wv_t = []
            for t_ in range(2):
                t = constp.tile([128, 2, D_QK], FP8, tag=f"wk8{t_}")
                nc.gpsimd.dma_start(t[:], wk8_d[128 * t_ : 128 * (t_ + 1), 0:2, :])
                wk8_t.append(t)
            for l in range(4):
                t = constp.tile([128, D_V], F32R, tag=f"wv{l}")
                nc.gpsimd.dma_start(t[:], r(wv_d[128 * l : 128 * (l + 1), :]))
                wv_t.append(t)

            # wo resident: per (head, n-block) moving tiles [128, 512], bf16
            wo_t = [[None] * 4 for _ in range(NHC)]
            for h in range(NHC):
                for n in range(4):
                    t = wop.tile([128, 512], BF16, tag=f"wo{h}_{n}")
                    nc.gpsimd.dma_start(
                        t[:], wo_d[128 * h : 128 * (h + 1), 512 * n : 512 * (n + 1)]
                    )
                    wo_t[h][n] = t

            # persistent K state across chunks: fp8 DoubleRow-packed
            # [96, 2, S]; combined dim d = 96*slot + p covers
            # [k_nope(128); k_rope(64)] = 192 rows.
            k8 = kfp.tile([96, 2, S], FP8, tag="k8")
            v_t = [
                vp.tile([128, D_V], BF16, tag=f"v{i}", name=f"v{i}")
                for i in range(NKT_TOT)
            ]

            for rep in range(reps):
              # W1 resident for the whole rep (re-DMA'd once per rep):
              # lora columns in bf16, query columns fp8 DoubleRow-packed
              w1_t = {}
              for si in range(2):
                  for ht in range(16):
                      t = w1p.tile(
                          [128, 256], BF16, tag=f"w1_{ht}_{si}", bufs=1,
                          name=f"w1_{ht}_{si}_{rep}",
                      )
                      eng = nc.scalar if ht < 10 else nc.sync
                      eng.dma_start(
                          t[:],
                          w1_d[128 * ht : 128 * (ht + 1), 256 * si : 256 * (si + 1)],
                      )
                      w1_t[(ht, si)] = t
              w1q8_t = []
              for ht in range(16):
                  t = w1p.tile(
                      [64, 2, NHC * D_QK], FP8, tag=f"w1q8_{ht}", bufs=1,
                      name=f"w1q8_{ht}_{rep}",
                  )
                  eng = nc.scalar if ht < 10 else nc.sync
                  eng.dma_start(t[:], w1q8_d[64 * ht : 64 * (ht + 1), 0:2, :])
                  w1q8_t.append(t)
              # full-width rope tables, once per rep
              cosf = ropep.tile([128, S], BF16, tag="cos", name=f"cos_{rep}")
              nc.gpsimd.dma_start(cosf[:], cos_d[:])
              sinf = ropep.tile([128, S], BF16, tag="sin", name=f"sin_{rep}")
              nc.gpsimd.dma_start(sinf[:], sin_d[:])

              for c in range(NCHUNK):
                s0 = SC * c

                # ---------------- phase A: C1 = X @ W1 (transposed) --------
                x_t = []
                x8_t = []
                for ht in range(16):
                    t = xp.tile([128, SC], BF16, tag=f"x{ht}")
                    nc.sync.dma_start(
                        t[:], xt_d[128 * ht : 128 * (ht + 1), s0 : s0 + SC]
                    )
                    x_t.append(t)
                    t8 = xp.tile([64, 2, SC], FP8, tag=f"x8{ht}", bufs=2,
                                  name=f"x8{ht}_{c}")
                    nc.sync.dma_start(
                        t8[:], xt8_d[64 * ht : 64 * (ht + 1), 0:2, s0 : s0 + SC]
                    )
                    x8_t.append(t8)

                q8_t = []
                ckv_t = []
                ckv8_t = [
                    ckvp.tile([128, 2, SC], FP8, tag=f"ckv8_{t_}",
                              name=f"ckv8_{t_}_{c}")
                    for t_ in range(2)
                ]
                qx1_ps = qx2_ps = None
                for j in range(10):
                    ps = mmp.tile([128, SC], F32, tag="mm")
                    if j < 6:
                        # query columns: fp8 DoubleRow at 0.5 cyc/row
                        for ht in range(16):
                            nc.tensor.matmul(
                                ps[:],
                                w1q8_t[ht][0:64, 0:2, 128 * j : 128 * (j + 1)],
                                x8_t[ht][0:64, 0:2, :],
                                start=(ht == 0),
                                stop=(ht == 15),
                                perf_mode=DR,
                            )
                    else:
                        si, off = ((j - 6) // 2, 128 * ((j - 6) % 2))
                        for ht in range(16):
                            nc.tensor.matmul(
                                ps[:],
                                w1_t[(ht, si)][:, off : off + 128],
                                x_t[ht][:],
                                start=(ht == 0),
                                stop=(ht == 15),
                            )
                    if j < 4:
                        # q_nope head j -> fp8 DR tile; slot0 (rows 0:96) is
                        # partition-aligned so DVE writes it directly
                        q8 = qnp.tile([96, 2, SC], FP8, tag=f"q8_{j}",
                                      name=f"q8_{j}_{c}")
                        nc.vector.tensor_copy(q8[0:96, 0, :], ps[0:96, :])
                        qh = qnp.tile([128, SC], FP8, tag=f"qh8_{j}")
                        nc.vector.tensor_copy(qh[96:128, :], ps[96:128, :])
                        nc.gpsimd.dma_start(q8[0:32, 1, :], qh[96:128, :])
                        q8_t.append(q8)
                    elif j == 4:
                        qx1_ps = ps
                    elif j == 5:
                        qx2_ps = ps
                    else:
                        t = ckvp.tile([128, SC], F32R, tag=f"ckv{j - 6}")
                        nc.scalar.copy(t[:], ps[:])
                        jj = j - 6
                        nc.scalar.copy(ckv8_t[jj // 2][0:128, jj % 2, :], ps[:])
                        ckv_t.append(t)

                # rope tables for this chunk
                cos_t = cosf[:, s0 : s0 + SC]
                sin_t = sinf[:, s0 : s0 + SC]

                # ---- Q rope (4 heads batched in 128 partitions) ----
                # All DVE ops full-tile (base partition 0); per-head row
                # extraction done with SB->SB DMAs (free to cross partitions).
                p1 = ropep.tile([128, SC], F32, tag="p1")
                t1 = ropep.tile([128, SC], F32, tag="t1")
                p2 = ropep.tile([128, SC], F32, tag="p2")
                t2 = ropep.tile([128, SC], F32, tag="t2")
                nc.vector.tensor_tensor(p1[:], qx1_ps[:], cos_t, AluOpType.mult)
                nc.vector.tensor_tensor(t1[:], qx2_ps[:], sin_t, AluOpType.mult)
                nc.vector.tensor_tensor(p2[:], qx2_ps[:], cos_t, AluOpType.mult)
                nc.vector.tensor_tensor(t2[:], qx1_ps[:], sin_t, AluOpType.mult)
                o1 = ropep.tile([128, SC], FP8, tag="o1")
                o2 = ropep.tile([128, SC], FP8, tag="o2")
                nc.vector.tensor_tensor(o1[:], p1[:], t1[:], AluOpType.subtract)
                nc.vector.tensor_tensor(o2[:], p2[:], t2[:], AluOpType.add)
                # finish q8 DR packing: rope rows into slot1
                for h in range(NHC):
                    sl = slice(32 * h, 32 * h + 32)
                    nc.gpsimd.dma_start(q8_t[h][32:64, 1, :], o1[sl, :])
                    nc.gpsimd.dma_start(q8_t[h][64:96, 1, :], o2[sl, :])

                # ---------------- K up-projection (fp8 DoubleRow) ---------
                ps_kn = mmp.tile([128, SC], F32, tag="mm")
                for t_ in range(2):
                    nc.tensor.matmul(
                        ps_kn[:],
                        wk8_t[t_][0:128, 0:2, 0:128],
                        ckv8_t[t_][0:128, 0:2, :],
                        start=(t_ == 0), stop=(t_ == 1), perf_mode=DR,
                    )
                # k8 slot0 (rows 0:96) partition-aligned: DVE writes direct
                nc.scalar.copy(k8[0:96, 0, s0 : s0 + SC], ps_kn[0:96, :])
                knh = ropep.tile([128, SC], FP8, tag="knh")
                nc.scalar.copy(knh[96:128, :], ps_kn[96:128, :])

                ps_kr = mmp.tile([64, SC], F32, tag="mm")
                for t_ in range(2):
                    nc.tensor.matmul(
                        ps_kr[:],
                        wk8_t[t_][0:128, 0:2, 128:192],
                        ckv8_t[t_][0:128, 0:2, :],
                        start=(t_ == 0), stop=(t_ == 1), perf_mode=DR,
                    )
                # K rope. kp = [x1*cos; x2*cos], kt = [x1*sin; x2*sin]
                # (cos/sin rows 0:32 == 32:64, so full-tile products work).
                # Swap kt halves via SB->SB DMA, then:
                #   k_rope[0:32]  = kp[0:32]  - kt_swap[0:32]   (= x1 cos - x2 sin)
                #   k_rope[32:64] = kp[32:64] + kt_swap[32:64]  (= x2 cos + x1 sin)
                kp = ropep.tile([64, SC], F32, tag="kp")
                kt_ = ropep.tile([64, SC], F32, tag="kt_")
                kts = ropep.tile([64, SC], F32, tag="kts")
                nc.vector.tensor_tensor(
                    kp[:], ps_kr[:], cosf[0:64, s0 : s0 + SC], AluOpType.mult
                )
                nc.vector.tensor_tensor(
                    kt_[:], ps_kr[:], sinf[0:64, s0 : s0 + SC], AluOpType.mult
                )
                nc.gpsimd.dma_start(kts[0:32, :], kt_[32:64, :])
                nc.gpsimd.dma_start(kts[32:64, :], kt_[0:32, :])
                kr8 = ropep.tile([64, SC], FP8, tag="kr8")
                nc.vector.tensor_tensor(
                    kr8[0:32, :], kp[0:32, :], kts[0:32, :], AluOpType.subtract
                )
                nc.vector.tensor_tensor(
                    kr8[32:64, :], kp[32:64, :], kts[32:64, :], AluOpType.add
                )
                # pack the remaining K slab rows into the fp8 DR layout
                nc.gpsimd.dma_start(k8[0:32, 1, s0 : s0 + SC], knh[96:128, :])
                nc.gpsimd.dma_start(k8[32:96, 1, s0 : s0 + SC], kr8[0:64, :])

                # ---------------- V up-projection ----------------
                # V^T = (CKV @ Wv)^T computed at full rate (free dim 512),
                # then PE-transposed back to V in 128x128 blocks.
                ps_vt = mmp.tile([128, SC], F32, tag="mm", name=f"vt_{c}")
                for l in range(4):
                    nc.tensor.matmul(
                        ps_vt[:], wv_t[l][:], ckv_t[l][:],
                        start=(l == 0), stop=(l == 3),
                    )
                vt_sb = ropep.tile([128, SC], F32R, tag="vt")
                nc.scalar.copy(vt_sb[:], ps_vt[:])
                for ss in range(4):
                    ps_v = mmp.tile([128, 128], F32R, tag="mm", name=f"vtr_{c}_{ss}")
                    nc.tensor.transpose(
                        ps_v[:], vt_sb[:, 128 * ss : 128 * (ss + 1)], eye_r[:]
                    )
                    nc.scalar.copy(v_t[4 * c + ss][:], ps_v[:])

                # ---------------- phase B: attention per head ----------------
                # softmax denominator: e-tiles are accumulated on DVE into
                # one (c==0) or two (c>0, alternating) SBUF accumulators;
                # a single ones-matmul per accumulator at the end of the
                # head does the cross-partition sum + broadcast. This keeps
                # the per-k-tile denominator work off the Tensor engine.
                o_norm = []
                for h in range(NHC):
                    nkt = 4 * c + 4
                    nacc = 1 if c == 0 else 2
                    dacc = [
                        ropep.tile([128, SC], F32R, tag=f"den{a}", bufs=1,
                                   name=f"den{a}_{c}_{h}")
                        for a in range(nacc)
                    ]
                    o_ps = op_.tile([128, SC], F32, tag="o", name=f"o_{c}_{h}")
                    for kt in range(nkt):
                        diag = kt >= 4 * c
                        p = (kt - 4 * c) * 128 if diag else 0
                        s_ps = mmp.tile([128, SC], F32, tag="mm")
                        nc.tensor.matmul(
                            s_ps[:, p:SC],
                            k8[0:96, 0:2, KT * kt : KT * (kt + 1)],
                            q8_t[h][0:96, 0:2, p:SC],
                            start=True,
                            stop=True,
                            perf_mode=DR,
                        )
                        e = ep.tile([128, SC], BF16, tag="e")
                        if diag:
                            tmp = ep.tile([128, 128], F32, tag="ediag", bufs=2,
                                          name=f"ediag_{c}_{h}_{kt}")
                            nc.scalar.activation(
                                tmp[:], s_ps[:, p : p + 128], EXP, scale=SCALE
                            )
                            nc.vector.tensor_tensor(
                                e[:, p : p + 128], tmp[:], tri_r[:], AluOpType.mult
                            )
                            if p + 128 < SC:
                                nc.scalar.activation(
                                    e[:, p + 128 : SC], s_ps[:, p + 128 : SC],
                                    EXP, scale=SCALE,
                                )
                        else:
                            nc.scalar.activation(e[:], s_ps[:], EXP, scale=SCALE)
                        acc = dacc[kt % nacc]
                        if kt < nacc:
                            # first write per accumulator: kt<nacc is always
                            # full-width (c==0,kt=0 has p=0; c>0 kt 0/1 are
                            # off-diagonal)
                            nc.vector.tensor_copy(acc[:], e[:])
                        else:
                            nc.vector.tensor_tensor(
                                acc[:, p:SC], acc[:, p:SC], e[:, p:SC],
                                AluOpType.add,
                            )
                        nc.tensor.matmul(
                            o_ps[:, p:SC],
                            v_t[kt][:],
                            e[:, p:SC],
                            start=(kt == 0),
                            stop=(kt == nkt - 1),
                        )
                    den_ps = denp.tile([128, SC], F32, tag="den")
                    for a in range(nacc):
                        nc.tensor.matmul(
                            den_ps[:], ones_r[:], dacc[a][:],
                            start=(a == 0), stop=(a == nacc - 1),
                        )
                    recip = ropep.tile([128, SC], F32, tag="recip",
                                       name=f"recip_{c}_{h}")
                    nc.vector.reciprocal(recip[:], den_ps[:])
                    on = onp.tile([128, SC], BF16, tag=f"on{h}")
                    nc.vector.tensor_tensor(on[:], o_ps[:], recip[:], AluOpType.mult)
                    o_norm.append(on)

                # ---------------- phase C: Y = O @ Wo (partial) -------------
                # y staged [128, 1024] (2 n-blocks) -> bigger output DMAs,
                # issued on the SWDGE ring to keep the HWDGE rings free
                for ss in range(4):
                    for np_ in range(2):
                        y_sb = yp.tile([128, 1024], BF16, tag="y", name=f"y_{c}_{ss}_{np_}")
                        for nn in range(2):
                            n = 2 * np_ + nn
                            y_ps = mmp.tile([128, 512], F32, tag="mm", name=f"yps_{c}_{ss}_{n}")
                            for h in range(NHC):
                                nc.tensor.matmul(
                                    y_ps[:],
                                    o_norm[h][:, 128 * ss : 128 * (ss + 1)],
                                    wo_t[h][n][:],
                                    start=(h == 0),
                                    stop=(h == NHC - 1),
                                )
                            nc.scalar.copy(y_sb[:, 512 * nn : 512 * (nn + 1)], y_ps[:])
                        nc.gpsimd.dma_start(
                            y_d[s0 + 128 * ss : s0 + 128 * (ss + 1),
                                1024 * np_ : 1024 * (np_ + 1)],
                            y_sb[:],
                        )

    nc.compile()
    return nc


def _pack_dr_rows(a, half=64):
    """[rows, cols] -> [rows//2, 2, cols]: within each 2*half-row block,
    row d = half*slot + p lands at (block*half + p, slot)."""
    rows, cols = a.shape
    t = a.reshape(rows // (2 * half), 2, half, cols)
    t = np.swapaxes(t, 1, 2)
    return np.ascontiguousarray(t.reshape(rows // 2, 2, cols))


def _host_inputs(hidden_states, Wqkv, Wk_up, Wv_up, Wo):
    """Build the 8 per-core input maps."""
    bf16 = mybir.dt.np(mybir.dt.bfloat16)
    f8 = mybir.dt.np(mybir.dt.float8e4)
    inv_freq = 1.0 / (ROPE_BASE ** (np.arange(0, D_ROPE, 2, dtype=np.float32) / D_ROPE))
    t = np.arange(S, dtype=np.float32)
    freqs = np.outer(t, inv_freq)  # [S, 32]
    cosq = np.ascontiguousarray(np.tile(np.cos(freqs).T, (4, 1))).astype(bf16)
    sinq = np.ascontiguousarray(np.tile(np.sin(freqs).T, (4, 1))).astype(bf16)
    tri = np.triu(np.ones((128, 128), dtype=np.float32))
    eye = np.eye(128, dtype=np.float32)

    lora_cols = Wqkv[:, NH * D_QK :]  # [HID, LORA]
    in_maps = []
    per_g = {}
    for g in range(NKV):
        nopes, x1s, x2s = [], [], []
        for h in range(NHC):
            H = NHC * g + h
            base = H * D_QK
            nopes.append(Wqkv[:, base : base + D_NOPE])
            x1s.append(Wqkv[:, base + D_NOPE : base + D_NOPE + 32])
            x2s.append(Wqkv[:, base + D_NOPE + 32 : base + D_QK])
        w1 = np.ascontiguousarray(lora_cols).astype(bf16)
        w1q8 = _pack_dr_rows(
            np.concatenate(nopes + x1s + x2s, axis=1).astype(np.float32)
        ).astype(f8)
        wk8 = _pack_dr_rows(
            np.concatenate(
                [
                    Wk_up[:, g * D_QK : g * D_QK + D_NOPE],
                    Wk_up[:, g * D_QK + D_NOPE : g * D_QK + D_NOPE + 32],
                    Wk_up[:, g * D_QK + D_NOPE + 32 : (g + 1) * D_QK],
                ],
                axis=1,
            ).astype(np.float32),
            half=128,
        ).astype(f8)
        wv = np.ascontiguousarray(Wv_up[:, g * D_V : (g + 1) * D_V]).astype(np.float32)
        wo = np.ascontiguousarray(Wo[g * NHC * D_V : (g + 1) * NHC * D_V, :]).astype(
            bf16
        )
        per_g[g] = (w1, w1q8, wk8, wv, wo)

    xt_b = {}
    xt8_b = {}
    for b in range(B):
        xt_f32 = np.ascontiguousarray(hidden_states[b].T).astype(np.float32)
        xt_b[b] = xt_f32.astype(bf16)
        xt8_b[b] = _pack_dr_rows(xt_f32).astype(f8)

    for core in range(NCORES):
        b, g = core // NKV, core % NKV
        w1, w1q8, wk8, wv, wo = per_g[g]
        in_maps.append(
            {
                "xt": xt_b[b],
                "xt8": xt8_b[b],
                "w1": w1,
                "w1q8": w1q8,
                "wk8": wk8,
                "wv": wv,
                "wo": wo,
                "cosq": cosq,
                "sinq": sinq,
                "tri": tri,
                "eye": eye,
            }
        )
    return in_maps


def kernel(hidden_states, Wqkv, Wk_up, Wv_up, Wo):
    hidden_states = np.asarray(hidden_states, dtype=np.float32)
    Wqkv = np.asarray(Wqkv, dtype=np.float32)
    Wk_up = np.asarray(Wk_up, dtype=np.float32)
    Wv_up = np.asarray(Wv_up, dtype=np.float32)
    Wo = np.asarray(Wo, dtype=np.float32)

    if "nc" not in _PROGRAM_CACHE:
        _PROGRAM_CACHE["nc"] = _build_program()
    nc = _PROGRAM_CACHE["nc"]

    in_maps = _host_inputs(hidden_states, Wqkv, Wk_up, Wv_up, Wo)
    res = run_bass_kernel_spmd(nc, in_maps, list(range(NCORES)))

    out = np.zeros((B, S, HID), dtype=np.float32)
    for core in range(NCORES):
        b = core // NKV
        out[b] += res.results[core]["y"].astype(np.float32)
    return out


if __name__ == "__main__":
    rng = np.random.default_rng(0)
    hs = rng.standard_normal((B, S, HID)).astype(np.float32)
    wqkv = rng.standard_normal((HID, NH * D_QK + LORA)).astype(np.float32) * 0.02
    wk = rng.standard_normal((LORA, NKV * D_QK)).astype(np.float32) * 0.04
    wv = rng.standard_normal((LORA, NKV * D_V)).astype(np.float32) * 0.04
    wo = rng.standard_normal((NH * D_V, HID)).astype(np.float32) * 0.02
    y = kernel(hs, wqkv, wk, wv, wo)
    print("kernel output", y.shape, y.dtype, float(np.abs(y).max()))

